# revision 2
# baseline (speedup 1.0000x reference)
"""Continual-attention Trainium2 kernel (8 NeuronCores, SPMD).

Sharding: core c -> batch b = c//2, head-group g = c%2 (4 heads each).
Per (b,h) computes S^T[k,q] = K Q^T via PE (float32r), additive masks
(causal diag / per-batch test-train / test-chunk) accumulated into PSUM
with extra matmuls, exp on ScalarE with fused 1/sqrt(d) scale, then
O^T[d,q] (+ softmax denominator as a 65th row via a ones column in V)
accumulated on PE. Normalization + final transpose happen on host.
"""

import sys

sys.path.insert(0, "/opt/trn_rl_repo")

import numpy as np
import ml_dtypes

B, L, H, D = 4, 2048, 8, 64
TRAIN = 1536
TEST = L - TRAIN            # 512
NCH = 64                    # test chunks
CH = TEST // NCH            # 8
HPC = 4                     # heads per core
NCORES = 8
KT = L // 128               # 16 k-tiles
NEG = -1.0e30

LAST_RESULT = None          # BassKernelResults of the most recent run
_PROG = None                # cached compiled Bass program


def _split_multi_waits(nc, mybir):
    """This container's walrus accepts at most one semaphore wait per
    instruction; Tile's tail drains can carry several. Hoist extras onto
    NoOps inserted immediately before, on the same engine."""
    for f in nc.m.functions:
        for bb in f.blocks:
            insts = list(bb.instructions)
            out = []
            changed = False
            for inst in insts:
                si = inst.sync_info
                if si is not None and len(si.on_wait) > 1:
                    waits = list(si.on_wait)
                    for w in waits[:-1]:
                        nop = mybir.InstNoOp(
                            name=f"waitnop-{nc.next_id()}", ins=[], outs=[]
                        )
                        nop.engine = inst.engine
                        nop.sync_info = mybir.SyncInfo(on_wait=[w], on_update=[])
                        out.append(nop)
                    inst.sync_info = mybir.SyncInfo(
                        on_wait=[waits[-1]], on_update=list(si.on_update)
                    )
                    changed = True
                out.append(inst)
            if changed:
                bb.instructions = out


def _build_program():
    import concourse.bass as bass
    import concourse.mybir as mybir
    import concourse.tile as tile

    f32 = mybir.dt.float32
    bf16 = mybir.dt.bfloat16
    f32r = mybir.dt.float32r
    Exp = mybir.ActivationFunctionType.Exp

    nc = bass.Bass()

    qt_d = nc.dram_tensor("qt", [HPC, D, L], f32r, kind="ExternalInput")
    kt_d = nc.dram_tensor("kt", [HPC, D, L], f32r, kind="ExternalInput")
    vw_d = nc.dram_tensor("vw", [HPC, 128, KT * 65], f32r, kind="ExternalInput")
    mtt_d = nc.dram_tensor("mtt", [128, 12 * 512], bf16, kind="ExternalInput")
    ident_d = nc.dram_tensor("ident", [128, 128], bf16, kind="ExternalInput")
    mdiag_d = nc.dram_tensor("mdiag", [128, 128], bf16, kind="ExternalInput")
    mchunk_d = nc.dram_tensor("mchunk", [128, 128], bf16, kind="ExternalInput")
    ot_d = nc.dram_tensor("ot", [HPC, 65, L], f32, kind="ExternalOutput")

    with tile.TileContext(nc) as tc:
        with (
            tc.tile_pool(name="consts", bufs=1) as consts,
            tc.tile_pool(name="heads", bufs=2) as heads,
            tc.tile_pool(name="ptp", bufs=6) as ptp,
            tc.tile_pool(name="osbp", bufs=3) as osbp,
            tc.tile_pool(name="spp", bufs=3, space="PSUM") as spp,
            tc.tile_pool(name="avp", bufs=2, space="PSUM") as avp,
        ):
            ident_sb = consts.tile([128, 128], bf16)
            nc.sync.dma_start(out=ident_sb, in_=ident_d.ap())
            mdiag_sb = consts.tile([128, 128], bf16)
            nc.sync.dma_start(out=mdiag_sb, in_=mdiag_d.ap())
            mchunk_sb = consts.tile([128, 128], bf16)
            nc.sync.dma_start(out=mchunk_sb, in_=mchunk_d.ap())
            mtt_sb = consts.tile([128, 12 * 512], bf16)
            nc.sync.dma_start(out=mtt_sb, in_=mtt_d.ap())

            for h in range(HPC):
                qt_sb = heads.tile([D, L], f32r, tag="qt")
                nc.sync.dma_start(out=qt_sb, in_=qt_d.ap()[h])
                kt_sb = heads.tile([D, L], f32r, tag="kt")
                nc.sync.dma_start(out=kt_sb, in_=kt_d.ap()[h])
                vw_sb = heads.tile([128, KT, 65], f32r, tag="vw")
                nc.sync.dma_start(
                    out=vw_sb,
                    in_=vw_d.ap()[h].rearrange("p (t c) -> p t c", t=KT),
                )

                for gq in range(4):
                    av = avp.tile([128, 512], f32, tag="av")
                    kps = list(range(4 * (gq + 1))) if gq < 3 else list(range(16))
                    last_kp = kps[-1]
                    for kp in kps:
                        if kp <= 11:
                            start_q = 512 * gq
                            off = max(0, 128 * kp - start_q)
                            w = 512 - off
                        else:
                            off = 128 * (kp - 12)
                            w = 128
                        qs = 512 * gq + off

                        sp = spp.tile([128, 512], f32, tag="sp")
                        diag = (kp <= 11 and 128 * kp >= 512 * gq) or kp >= 12
                        tt = gq == 3 and kp <= 11
                        nc.tensor.matmul(
                            sp[:, off : off + w],
                            lhsT=kt_sb[:, 128 * kp : 128 * kp + 128],
                            rhs=qt_sb[:, qs : qs + w],
                            start=True,
                            stop=not (diag or tt),
                            skip_group_check=True,
                        )
                        if diag:
                            mask_sb = mdiag_sb if kp <= 11 else mchunk_sb
                            nc.tensor.matmul(
                                sp[:, off : off + 128],
                                lhsT=ident_sb,
                                rhs=mask_sb,
                                start=False,
                                stop=not tt,
                                skip_group_check=True,
                            )
                        if tt:
                            nc.tensor.matmul(
                                sp[:, 0:512],
                                lhsT=ident_sb,
                                rhs=mtt_sb[:, 512 * kp : 512 * kp + 512],
                                start=False,
                                stop=True,
                                skip_group_check=True,
                            )

                        pt = ptp.tile([128, 512], f32r, tag="pt")
                        nc.scalar.activation(
                            pt[:, 0:w], sp[:, off : off + w], Exp, scale=0.125
                        )

                        nc.tensor.matmul(
                            av[:65, off : off + w],
                            lhsT=vw_sb[:, kp, :],
                            rhs=pt[:, 0:w],
                            start=(kp == 0),
                            stop=(kp == last_kp),
                            skip_group_check=True,
                        )

                    osb = osbp.tile([65, 512], f32)
                    nc.vector.tensor_copy(osb, av[:65, :])
                    nc.sync.dma_start(
                        out=ot_d.ap()[h][:, 512 * gq : 512 * gq + 512], in_=osb
                    )

    import concourse.mybir as mybir_mod

    _split_multi_waits(nc, mybir_mod)
    return nc


def _host_inputs(queries, keys, values, attach):
    """Build per-core input maps (host-side layout prep)."""
    bf = ml_dtypes.bfloat16
    p = np.arange(128)
    f = np.arange(128)
    ident = (p[:, None] == f[None, :]).astype(np.float32)
    mdiag = np.where(f[None, :] >= p[:, None], 0.0, NEG).astype(np.float32)
    mchunk = np.where(
        (p[:, None] // CH == f[None, :] // CH) & (p[:, None] <= f[None, :]),
        0.0,
        NEG,
    ).astype(np.float32)

    in_maps = []
    for c in range(NCORES):
        b, g = divmod(c, 2)
        hs = slice(HPC * g, HPC * (g + 1))
        q = queries[b][:, hs, :]          # [L, 4, D]
        k = keys[b][:, hs, :]
        v = values[b][:, hs, :]
        qt = np.ascontiguousarray(q.transpose(1, 2, 0))   # [4, D, L]
        kt = np.ascontiguousarray(k.transpose(1, 2, 0))
        vw = np.empty((HPC, L, 65), np.float32)
        vw[:, :, :64] = v.transpose(1, 0, 2)
        vw[:, :, 64] = 1.0
        # [4, L, 65] -> [4, 128, KT*65] with row p holding tile-chunks
        vw = np.ascontiguousarray(
            vw.reshape(HPC, KT, 128, 65).transpose(0, 2, 1, 3).reshape(HPC, 128, KT * 65)
        )
        kg = (np.arange(12)[:, None] * 128 + np.arange(128)[None, :])  # [12,128]
        thr = attach[b][np.arange(TEST) // CH]                          # [512]
        mtt = np.where(kg[:, :, None] <= thr[None, None, :], 0.0, NEG)  # [12,128,512]
        mtt = np.ascontiguousarray(mtt.transpose(1, 0, 2).reshape(128, 12 * 512))
        in_maps.append(
            {
                "qt": qt.astype(np.float32),
                "kt": kt.astype(np.float32),
                "vw": vw,
                "mtt": mtt.astype(bf),
                "ident": ident.astype(bf),
                "mdiag": mdiag.astype(bf),
                "mchunk": mchunk.astype(bf),
            }
        )
    return in_maps


def kernel(queries, keys, values, attach_test_after, train_len):
    global LAST_RESULT, _PROG
    import os

    queries = np.asarray(queries, dtype=np.float32)
    keys = np.asarray(keys, dtype=np.float32)
    values = np.asarray(values, dtype=np.float32)
    attach = np.asarray(attach_test_after).astype(np.int64)
    tl = int(np.asarray(train_len))
    assert queries.shape == (B, L, H, D), queries.shape
    assert tl == TRAIN and attach.shape == (B, NCH)

    from concourse.bass_utils import run_bass_kernel_spmd

    if _PROG is None:
        _PROG = _build_program()

    in_maps = _host_inputs(queries, keys, values, attach)
    trace = bool(int(os.environ.get("KERNEL_TRACE", "0")))
    res = run_bass_kernel_spmd(
        _PROG, in_maps, core_ids=list(range(NCORES)), trace=trace
    )
    LAST_RESULT = res

    out = np.empty((B, L, H * D), np.float32)
    for c in range(NCORES):
        b, g = divmod(c, 2)
        ot = res.results[c]["ot"]                     # [4, 65, L]
        o = ot[:, :64, :] / ot[:, 64:65, :]           # [4, 64, L]
        out[b, :, 256 * g : 256 * (g + 1)] = (
            o.transpose(2, 0, 1).reshape(L, HPC * D)
        )
    return out


# revision 3
# speedup vs baseline: 1.2035x; 1.2035x over previous
"""Continual-attention Trainium2 kernel (8 NeuronCores, SPMD).

Sharding: core c -> batch b = c//2, head-group g = c%2 (4 heads each).
Per (b,h) computes S^T[k,q] = K Q^T via PE (float32r), additive masks
(causal diag / per-batch test-train / test-chunk) accumulated into PSUM
with extra matmuls, exp on ScalarE with fused 1/sqrt(d) scale, then
O^T[d,q] (+ softmax denominator as a 65th row via a ones column in V)
accumulated on PE. Normalization + final transpose happen on host.
"""

import sys

sys.path.insert(0, "/opt/trn_rl_repo")

import numpy as np
import ml_dtypes

B, L, H, D = 4, 2048, 8, 64
TRAIN = 1536
TEST = L - TRAIN            # 512
NCH = 64                    # test chunks
CH = TEST // NCH            # 8
HPC = 4                     # heads per core
NCORES = 8
KT = L // 128               # 16 k-tiles
NEG = -60000.0  # exp(NEG*0.125) == 0; fits fp16

LAST_RESULT = None          # BassKernelResults of the most recent run
_PROG = None                # cached compiled Bass program


def _split_multi_waits(nc, mybir):
    """This container's walrus accepts at most one semaphore wait per
    instruction; Tile's tail drains can carry several. Hoist extras onto
    NoOps inserted immediately before, on the same engine."""
    for f in nc.m.functions:
        for bb in f.blocks:
            insts = list(bb.instructions)
            out = []
            changed = False
            for inst in insts:
                si = inst.sync_info
                if si is not None and len(si.on_wait) > 1:
                    waits = list(si.on_wait)
                    for w in waits[:-1]:
                        nop = mybir.InstNoOp(
                            name=f"waitnop-{nc.next_id()}", ins=[], outs=[]
                        )
                        nop.engine = inst.engine
                        nop.sync_info = mybir.SyncInfo(on_wait=[w], on_update=[])
                        out.append(nop)
                    inst.sync_info = mybir.SyncInfo(
                        on_wait=[waits[-1]], on_update=list(si.on_update)
                    )
                    changed = True
                out.append(inst)
            if changed:
                bb.instructions = out


def _build_program():
    import concourse.bass as bass
    import concourse.mybir as mybir
    import concourse.tile as tile

    f32 = mybir.dt.float32
    bf16 = mybir.dt.bfloat16
    f32r = mybir.dt.float32r
    fp16 = mybir.dt.float16
    Exp = mybir.ActivationFunctionType.Exp

    nc = bass.Bass()

    qt_d = nc.dram_tensor("qt", [HPC, D, L], fp16, kind="ExternalInput")
    kt_d = nc.dram_tensor("kt", [HPC, D, L], fp16, kind="ExternalInput")
    vw_d = nc.dram_tensor("vw", [HPC, 128, KT * 65], fp16, kind="ExternalInput")
    mtt_d = nc.dram_tensor("mtt", [128, 12 * 512], fp16, kind="ExternalInput")
    ident_d = nc.dram_tensor("ident", [128, 128], fp16, kind="ExternalInput")
    mdiag_d = nc.dram_tensor("mdiag", [128, 128], fp16, kind="ExternalInput")
    mchunk_d = nc.dram_tensor("mchunk", [128, 128], fp16, kind="ExternalInput")
    ot_d = nc.dram_tensor("ot", [HPC, 65, L], f32, kind="ExternalOutput")

    with tile.TileContext(nc) as tc:
        with (
            tc.tile_pool(name="consts", bufs=1) as consts,
            tc.tile_pool(name="heads", bufs=2) as heads,
            tc.tile_pool(name="ptp", bufs=6) as ptp,
            tc.tile_pool(name="osbp", bufs=3) as osbp,
            tc.tile_pool(name="spp", bufs=3, space="PSUM") as spp,
            tc.tile_pool(name="avp", bufs=2, space="PSUM") as avp,
        ):
            ident_sb = consts.tile([128, 128], fp16)
            nc.sync.dma_start(out=ident_sb, in_=ident_d.ap())
            mdiag_sb = consts.tile([128, 128], fp16)
            nc.sync.dma_start(out=mdiag_sb, in_=mdiag_d.ap())
            mchunk_sb = consts.tile([128, 128], fp16)
            nc.sync.dma_start(out=mchunk_sb, in_=mchunk_d.ap())
            mtt_sb = consts.tile([128, 12 * 512], fp16)
            nc.sync.dma_start(out=mtt_sb, in_=mtt_d.ap())

            for h in range(HPC):
                qt_sb = heads.tile([D, L], fp16, tag="qt")
                nc.sync.dma_start(out=qt_sb, in_=qt_d.ap()[h])
                kt_sb = heads.tile([D, L], fp16, tag="kt")
                nc.sync.dma_start(out=kt_sb, in_=kt_d.ap()[h])
                vw_sb = heads.tile([128, KT, 65], fp16, tag="vw")
                nc.sync.dma_start(
                    out=vw_sb,
                    in_=vw_d.ap()[h].rearrange("p (t c) -> p t c", t=KT),
                )

                for gq in range(4):
                    av = avp.tile([128, 512], f32, tag="av")
                    kps = list(range(4 * (gq + 1))) if gq < 3 else list(range(16))
                    last_kp = kps[-1]
                    for kp in kps:
                        if kp <= 11:
                            start_q = 512 * gq
                            off = max(0, 128 * kp - start_q)
                            w = 512 - off
                        else:
                            off = 128 * (kp - 12)
                            w = 128
                        qs = 512 * gq + off

                        sp = spp.tile([128, 512], f32, tag="sp")
                        diag = (kp <= 11 and 128 * kp >= 512 * gq) or kp >= 12
                        tt = gq == 3 and kp <= 11
                        nc.tensor.matmul(
                            sp[:, off : off + w],
                            lhsT=kt_sb[:, 128 * kp : 128 * kp + 128],
                            rhs=qt_sb[:, qs : qs + w],
                            start=True,
                            stop=not (diag or tt),
                            skip_group_check=True,
                        )
                        if diag:
                            mask_sb = mdiag_sb if kp <= 11 else mchunk_sb
                            nc.tensor.matmul(
                                sp[:, off : off + 128],
                                lhsT=ident_sb,
                                rhs=mask_sb,
                                start=False,
                                stop=not tt,
                                skip_group_check=True,
                            )
                        if tt:
                            nc.tensor.matmul(
                                sp[:, 0:512],
                                lhsT=ident_sb,
                                rhs=mtt_sb[:, 512 * kp : 512 * kp + 512],
                                start=False,
                                stop=True,
                                skip_group_check=True,
                            )

                        pt = ptp.tile([128, 512], fp16, tag="pt")
                        nc.scalar.activation(
                            pt[:, 0:w], sp[:, off : off + w], Exp, scale=0.125
                        )

                        nc.tensor.matmul(
                            av[:65, off : off + w],
                            lhsT=vw_sb[:, kp, :],
                            rhs=pt[:, 0:w],
                            start=(kp == 0),
                            stop=(kp == last_kp),
                            skip_group_check=True,
                        )

                    osb = osbp.tile([65, 512], f32)
                    nc.vector.tensor_copy(osb, av[:65, :])
                    nc.sync.dma_start(
                        out=ot_d.ap()[h][:, 512 * gq : 512 * gq + 512], in_=osb
                    )

    import concourse.mybir as mybir_mod

    _split_multi_waits(nc, mybir_mod)
    return nc


def _host_inputs(queries, keys, values, attach):
    """Build per-core input maps (host-side layout prep)."""
    f16 = np.float16
    p = np.arange(128)
    f = np.arange(128)
    ident = (p[:, None] == f[None, :]).astype(np.float32)
    mdiag = np.where(f[None, :] >= p[:, None], 0.0, NEG).astype(np.float32)
    mchunk = np.where(
        (p[:, None] // CH == f[None, :] // CH) & (p[:, None] <= f[None, :]),
        0.0,
        NEG,
    ).astype(np.float32)

    in_maps = []
    for c in range(NCORES):
        b, g = divmod(c, 2)
        hs = slice(HPC * g, HPC * (g + 1))
        q = queries[b][:, hs, :]          # [L, 4, D]
        k = keys[b][:, hs, :]
        v = values[b][:, hs, :]
        qt = np.ascontiguousarray(q.transpose(1, 2, 0))   # [4, D, L]
        kt = np.ascontiguousarray(k.transpose(1, 2, 0))
        vw = np.empty((HPC, L, 65), np.float32)
        vw[:, :, :64] = v.transpose(1, 0, 2)
        vw[:, :, 64] = 1.0
        # [4, L, 65] -> [4, 128, KT*65] with row p holding tile-chunks
        vw = np.ascontiguousarray(
            vw.reshape(HPC, KT, 128, 65).transpose(0, 2, 1, 3).reshape(HPC, 128, KT * 65)
        )
        kg = (np.arange(12)[:, None] * 128 + np.arange(128)[None, :])  # [12,128]
        thr = attach[b][np.arange(TEST) // CH]                          # [512]
        mtt = np.where(kg[:, :, None] <= thr[None, None, :], 0.0, NEG)  # [12,128,512]
        mtt = np.ascontiguousarray(mtt.transpose(1, 0, 2).reshape(128, 12 * 512))
        in_maps.append(
            {
                "qt": qt.astype(f16),
                "kt": kt.astype(f16),
                "vw": vw.astype(f16),
                "mtt": mtt.astype(f16),
                "ident": ident.astype(f16),
                "mdiag": mdiag.astype(f16),
                "mchunk": mchunk.astype(f16),
            }
        )
    return in_maps


def kernel(queries, keys, values, attach_test_after, train_len):
    global LAST_RESULT, _PROG
    import os

    queries = np.asarray(queries, dtype=np.float32)
    keys = np.asarray(keys, dtype=np.float32)
    values = np.asarray(values, dtype=np.float32)
    attach = np.asarray(attach_test_after).astype(np.int64)
    tl = int(np.asarray(train_len))
    assert queries.shape == (B, L, H, D), queries.shape
    assert tl == TRAIN and attach.shape == (B, NCH)

    from concourse.bass_utils import run_bass_kernel_spmd

    if _PROG is None:
        _PROG = _build_program()

    in_maps = _host_inputs(queries, keys, values, attach)
    trace = bool(int(os.environ.get("KERNEL_TRACE", "0")))
    res = run_bass_kernel_spmd(
        _PROG, in_maps, core_ids=list(range(NCORES)), trace=trace
    )
    LAST_RESULT = res

    out = np.empty((B, L, H * D), np.float32)
    for c in range(NCORES):
        b, g = divmod(c, 2)
        ot = res.results[c]["ot"]                     # [4, 65, L]
        o = ot[:, :64, :] / ot[:, 64:65, :]           # [4, 64, L]
        out[b, :, 256 * g : 256 * (g + 1)] = (
            o.transpose(2, 0, 1).reshape(L, HPC * D)
        )
    return out
